# revision 7
# baseline (speedup 1.0000x reference)
"""Trainium2 Bass kernel for nn_BaselineProt (embedding_lookup).

The reference computes, per drug-pair sample:
    multihot(drug) @ W0.T  ==  sum of W0 columns at the drug's (deduped)
    target proteins -- i.e. an embedding-table gather/sum, followed by a
    tiny MLP tower on each leg and a dot product between the two legs.

Structure (8 NeuronCores, data-parallel):
  Launch A: drugs sharded 500/core (padded to 512). Each core issues 4
      large dma_gathers (4096 idxs each, all on ONE SWDGE queue so they
      complete in FIFO order) of 512B bf16 rows of the transposed W0
      table (dups remapped to a zero row to preserve `.set` semantics)
      and tree-reduces each [128, 32, 256] tile to an E shard row block
      while later gathers still drain.
  Host:     concatenates the 8 E shards + 32 (cell + b0) rows into one
      lookup table E_ext [4128, 256] (pure data movement).
  Launch B: batch sharded 1024 samples/core, split in 2 groups of 512.
      Per group one transpose-mode gather pulls, in block layout,
      [E[d0] x512 | E[d1] x512 | cellb x512] feature-major; DVE adds
      e-leg + cellb (contiguous), ACT applies ReLU (b0 pre-folded into
      cellb), two matmul layers (W1, W2) and a ones-matmul pair-dot
      produce the [1024] outputs per core.
"""

import os

os.environ.setdefault("JAX_PLATFORMS", "")

import numpy as np
import ml_dtypes

import concourse.bacc as bacc
import concourse.mybir as mybir
from concourse.tile import TileContext
from concourse import library_config
from concourse.bass_utils import run_bass_kernel_spmd

# Problem constants (hardcoded per harness contract).
B = 8192            # samples
P = 19000           # proteins
C = 32              # cell lines
D = 4000            # drugs
T = 32              # targets per drug
F = 256             # first hidden dim
H1 = 128            # second hidden dim
H2 = 64             # output dim per tower

NCORES = 8
DRUGS_PER_CORE = D // NCORES          # 500
DRUGS_PAD = 512                       # per-core padded drug count
SAMPLES_PER_CORE = B // NCORES        # 1024
ZROW = P + C                          # zero row in the W0T table (19032)
TAB_ROWS = ZROW + 8                   # pad table rows to 19040
E_ROWS = NCORES * DRUGS_PAD           # 4096 rows of E
EXT_ROWS = E_ROWS + C                 # + 32 cell rows = 4128
NI_A = DRUGS_PAD * T                  # 16384 gather idxs per core, launch A
NI_B = 3 * SAMPLES_PER_CORE           # 3072 gather idxs per core, launch B
N_SUB = 4                             # launch A sub-batches of 128 drugs
NI_S = NI_A // N_SUB                  # 4096 idxs per launch-A gather
GRP = 2                               # launch B sample groups
SG = SAMPLES_PER_CORE // GRP          # 512 samples per group
NI_G = 3 * SG                         # 1536 idxs per launch-B gather
NQ = 4                                # SWDGE queues

_BF16 = mybir.dt.bfloat16
_F32 = mybir.dt.float32
_I16 = mybir.dt.int16

_cache = {}


def _wrap_idx(flat):
    """Flat gather order -> the [128, n/16] int16 SBUF layout dma_gather
    expects (idx i at partition i%16, slot i//16; replicated to all 8 Q7
    core slices)."""
    n = flat.shape[0]
    assert n % 16 == 0
    arr = flat.astype(np.int16).reshape(n // 16, 16).T.copy()
    return np.tile(arr, (8, 1))


def _build_kernel_a():
    nc = bacc.Bacc("TRN2", target_bir_lowering=True, num_swdge_queues=NQ)
    tab = nc.dram_tensor("tab", [TAB_ROWS, F], _BF16, kind="ExternalInput")
    idxs = nc.dram_tensor("idxs", [128, NI_A // 16], _I16, kind="ExternalInput")
    e_out = nc.dram_tensor("e_out", [DRUGS_PAD, F], _BF16, kind="ExternalOutput")

    with TileContext(nc) as tc:
        nc.gpsimd.load_library(library_config.mlp)
        with (
            tc.tile_pool(name="idx", bufs=1) as ip,
            tc.tile_pool(name="g", bufs=1) as gp,
            tc.tile_pool(name="e", bufs=2) as ep,
        ):
            idx_t = ip.tile([128, NI_A // 16], _I16)
            nc.sync.dma_start(out=idx_t[:, :], in_=idxs[:, :])
            # one big gather per 128-drug sub-batch, ALL on queue 0: the
            # queue drains FIFO, so gather b completes while b+1 streams
            # and the DVE tree for b overlaps the remaining drain.
            gs = []
            for b in range(N_SUB):
                g = gp.tile([128, T, F], _BF16, tag=f"g{b}")
                nc.gpsimd.dma_gather(
                    g[:, :, :],
                    tab[:],
                    idx_t[:, b * (NI_S // 16):(b + 1) * (NI_S // 16)],
                    NI_S, NI_S, F,
                    single_packet=False, queue_num=0,
                )
                gs.append(g)
            for b in range(N_SUB):
                g = gs[b]
                w = T // 2
                while w >= 1:
                    out_ap = g[:, 0:w, :]
                    if w == 1:
                        e_strip = ep.tile([128, F], _BF16, tag=f"e{b % 2}")
                        out_ap = e_strip[:, :].rearrange("p (a f) -> p a f", a=1)
                    nc.vector.tensor_tensor(
                        out=out_ap,
                        in0=g[:, 0:w, :],
                        in1=g[:, w:2 * w, :],
                        op=mybir.AluOpType.add,
                    )
                    w //= 2
                nc.sync.dma_start(
                    out=e_out[b * 128:(b + 1) * 128, :], in_=e_strip[:, :]
                )
    nc.compile()
    return nc


def _build_kernel_b():
    nc = bacc.Bacc("TRN2", target_bir_lowering=True, num_swdge_queues=NQ)
    etab = nc.dram_tensor("etab", [EXT_ROWS, F], _BF16, kind="ExternalInput")
    idxs = nc.dram_tensor("idxs", [128, NI_B // 16], _I16, kind="ExternalInput")
    w1t = nc.dram_tensor("w1t", [F, H1], _BF16, kind="ExternalInput")
    w2t = nc.dram_tensor("w2t", [H1, H2], _BF16, kind="ExternalInput")
    b1t = nc.dram_tensor("b1t", [128, 1], _F32, kind="ExternalInput")
    b2t = nc.dram_tensor("b2t", [64, 1], _F32, kind="ExternalInput")
    y = nc.dram_tensor("y", [1, SAMPLES_PER_CORE], _F32, kind="ExternalOutput")

    S = SAMPLES_PER_CORE                      # 1024
    L = 2 * S                                 # 2048 legs
    TN = 512                                  # matmul moving-tile width
    with TileContext(nc) as tc:
        nc.gpsimd.load_library(library_config.mlp)
        with (
            tc.tile_pool(name="const", bufs=1) as cp,
            tc.tile_pool(name="act", bufs=1) as ap,
            tc.tile_pool(name="ps", bufs=2, space="PSUM") as pp,
        ):
            idx_t = cp.tile([128, NI_B // 16], _I16)
            nc.sync.dma_start(out=idx_t[:, :], in_=idxs[:, :])
            # W1T is [256, H1]; SBUF partition dim is 128 -> [128, 2, H1]
            w1_t = cp.tile([128, 2, H1], _BF16, tag="w1")
            nc.sync.dma_start(
                out=w1_t[:, :, :],
                in_=w1t.ap().rearrange("(c p) h -> p c h", p=128),
            )
            w2_t = cp.tile([128, H2], _BF16, tag="w2")
            nc.sync.dma_start(out=w2_t[:, :], in_=w2t[:, :])
            b1_t = cp.tile([128, 1], _F32, tag="b1")
            nc.sync.dma_start(out=b1_t[:, :], in_=b1t[:, :])
            b2_t = cp.tile([64, 1], _F32, tag="b2")
            nc.sync.dma_start(out=b2_t[:, :], in_=b2t[:, :])
            ones = cp.tile([64, 1], _F32, tag="ones")
            nc.vector.memset(ones[:, :], 1.0)
            zero_b = cp.tile([128, 1], _F32, tag="zb")
            nc.vector.memset(zero_b[:, :], 0.0)

            # fused gather, 6 transpose-mode gathers of 512 idxs (32
            # sixteen-idx packets each -- large single-packet transpose
            # gathers hang the SDMA), per 512-sample group h in role
            # order [cellb, E[d0], E[d1]]; all on queue 0 -> FIFO
            # completion so group-h compute overlaps group-h+1 drain.
            xg = [[None] * 3 for _ in range(GRP)]
            for h in range(GRP):
                for r in range(3):
                    xt = ap.tile([128, 2, SG], _BF16, tag=f"xt{h}{r}")
                    nc.gpsimd.dma_gather(
                        xt[:, :, :], etab[:],
                        idx_t[:, (3 * h + r) * (SG // 16):
                              (3 * h + r + 1) * (SG // 16)],
                        SG, SG, F,
                        transpose=True, single_packet=True, queue_num=0,
                    )
                    xg[h][r] = xt

            h0 = ap.tile([128, 2, L], _BF16, tag="h0")
            h1 = ap.tile([128, L], _BF16, tag="h1")
            h2 = ap.tile([64, L], _F32, tag="h2")
            prod = ap.tile([64, S], _F32, tag="prod")
            out_sb = ap.tile([1, S], _F32, tag="out")
            for h in range(GRP):
                hb = h * 2 * SG
                for leg in range(2):
                    nt = 2 * h + leg
                    # pre-activation: e-leg + cellb (b0 folded into cellb)
                    nc.vector.tensor_tensor(
                        out=h0[:, :, hb + leg * SG:hb + (leg + 1) * SG],
                        in0=xg[h][1 + leg][:, :, :],
                        in1=xg[h][0][:, :, :],
                        op=mybir.AluOpType.add,
                    )
                    for c in range(2):
                        nc.scalar.activation(
                            h0[:, c, hb + leg * SG:hb + (leg + 1) * SG],
                            h0[:, c, hb + leg * SG:hb + (leg + 1) * SG],
                            mybir.ActivationFunctionType.Relu,
                            bias=zero_b[:, 0:1], scale=1.0,
                        )
                    ps1 = pp.tile([128, TN], _F32, tag="ps1")
                    for c in range(2):
                        nc.tensor.matmul(
                            ps1[:, :], w1_t[:, c, :],
                            h0[:, c, nt * TN:(nt + 1) * TN],
                            start=(c == 0), stop=(c == 1),
                        )
                    nc.scalar.activation(
                        h1[:, nt * TN:(nt + 1) * TN], ps1[:, :],
                        mybir.ActivationFunctionType.Relu,
                        bias=b1_t[:, 0:1], scale=1.0,
                    )
                    ps2 = pp.tile([64, TN], _F32, tag="ps2")
                    nc.tensor.matmul(
                        ps2[:, :], w2_t[:, :], h1[:, nt * TN:(nt + 1) * TN],
                        start=True, stop=True,
                    )
                    # h2 = ps2 + b2 on DVE (keeps scalar engine all-Relu,
                    # avoiding activation-table reloads)
                    nc.vector.tensor_scalar(
                        out=h2[:, nt * TN:(nt + 1) * TN],
                        in0=ps2[:, :],
                        scalar1=b2_t[:, 0:1], scalar2=None,
                        op0=mybir.AluOpType.add,
                    )
                # pair product: leg0 block x leg1 block of this group
                nc.vector.tensor_tensor(
                    out=prod[:, h * SG:(h + 1) * SG],
                    in0=h2[:, hb:hb + SG],
                    in1=h2[:, hb + SG:hb + 2 * SG],
                    op=mybir.AluOpType.mult,
                )
                ps3 = pp.tile([1, SG], _F32, tag="ps3")
                nc.tensor.matmul(
                    ps3[:, :], ones[:, :], prod[:, h * SG:(h + 1) * SG],
                    start=True, stop=True,
                )
                nc.vector.tensor_copy(
                    out_sb[:, h * SG:(h + 1) * SG], ps3[:, :]
                )
            nc.sync.dma_start(out=y[:, :], in_=out_sb[:, :])
    nc.compile()
    return nc


def _get_kernels():
    if "a" not in _cache:
        _cache["a"] = _build_kernel_a()
    if "b" not in _cache:
        _cache["b"] = _build_kernel_b()
    return _cache["a"], _cache["b"]


def _prep(drug_pairs, cell_lines, drug_targets, W0, b0, W1, b1, W2, b2):
    """Host-side data layout: shard, transpose, cast, build gather indices."""
    dt = np.asarray(drug_targets, dtype=np.int64)                  # [D, T]
    # dedup per row (reference uses .set -> dup targets count once)
    dup = (dt[:, :, None] == dt[:, None, :]) & (
        np.arange(T)[None, :, None] > np.arange(T)[None, None, :]
    )
    idx = np.where(dup.any(-1), ZROW, dt).astype(np.int32)          # [D, T]

    # W0T table: [P+C rows, F] bf16 + zero row + pad
    w0t = np.zeros((TAB_ROWS, F), dtype=ml_dtypes.bfloat16)
    w0t[: P + C] = np.asarray(W0, np.float32).T.astype(ml_dtypes.bfloat16)

    # launch A per-core gather index arrays
    idx_a = []
    for c in range(NCORES):
        rows = np.full((DRUGS_PAD, T), ZROW, np.int32)
        rows[:DRUGS_PER_CORE] = idx[c * DRUGS_PER_CORE:(c + 1) * DRUGS_PER_CORE]
        # flat j = b*4096 + t*128 + p  ->  drug 128b+p, target t
        flat = rows.reshape(N_SUB, 128, T).transpose(0, 2, 1).reshape(-1)
        idx_a.append(_wrap_idx(flat))

    # launch B per-core index arrays (built against E_ext layout), block
    # layout per 512-sample group: [e0 x512 | e1 x512 | cellb x512]
    dp = np.asarray(drug_pairs, dtype=np.int64)                     # [B, 2]
    cl = np.asarray(cell_lines, dtype=np.int64)                     # [B]
    e_row = (dp // DRUGS_PER_CORE) * DRUGS_PAD + (dp % DRUGS_PER_CORE)
    cell_row = E_ROWS + cl
    idx_b = []
    for c in range(NCORES):
        sl = slice(c * SAMPLES_PER_CORE, (c + 1) * SAMPLES_PER_CORE)
        e0, e1, cr = e_row[sl, 0], e_row[sl, 1], cell_row[sl]
        parts = []
        for h in range(GRP):
            g = slice(h * SG, (h + 1) * SG)
            parts += [cr[g], e0[g], e1[g]]
        idx_b.append(_wrap_idx(np.concatenate(parts)))

    w1t = np.ascontiguousarray(
        np.asarray(W1, np.float32).T.astype(ml_dtypes.bfloat16))    # [F, H1]
    w2t = np.ascontiguousarray(
        np.asarray(W2, np.float32).T.astype(ml_dtypes.bfloat16))    # [H1, H2]
    b1t = np.asarray(b1, np.float32).reshape(128, 1).copy()
    b2t = np.asarray(b2, np.float32).reshape(64, 1).copy()
    # cellb rows: W0 cell columns + b0, so launch B needs no separate bias
    celltab = (
        np.asarray(W0, np.float32)[:, P:P + C].T
        + np.asarray(b0, np.float32)[None, :]
    ).astype(ml_dtypes.bfloat16)                                    # [C, F]
    return w0t, idx_a, idx_b, w1t, w2t, b1t, b2t, celltab


def _run(inputs, trace=False):
    nca, ncb = _get_kernels()
    w0t, idx_a, idx_b, w1t, w2t, b1t, b2t, celltab = _prep(**inputs)

    in_a = [{"tab": w0t, "idxs": idx_a[c]} for c in range(NCORES)]
    res_a = run_bass_kernel_spmd(
        nca, in_a, core_ids=list(range(NCORES)), trace=trace)

    e_ext = np.concatenate(
        [res_a.results[c]["e_out"] for c in range(NCORES)] + [celltab], axis=0
    )
    assert e_ext.shape == (EXT_ROWS, F)

    in_b = [
        {"etab": e_ext, "idxs": idx_b[c], "w1t": w1t, "w2t": w2t,
         "b1t": b1t, "b2t": b2t}
        for c in range(NCORES)
    ]
    res_b = run_bass_kernel_spmd(
        ncb, in_b, core_ids=list(range(NCORES)), trace=trace)
    _cache["res_a"], _cache["res_b"] = res_a, res_b

    out = np.concatenate(
        [res_b.results[c]["y"].reshape(-1) for c in range(NCORES)]
    ).astype(np.float32)
    times = (res_a.exec_time_ns, res_b.exec_time_ns)
    return out, times


def kernel(**inputs) -> np.ndarray:
    out, _ = _run(inputs, trace=False)
    return out


# revision 11
# speedup vs baseline: 1.9650x; 1.9650x over previous
"""Trainium2 Bass kernel for nn_BaselineProt (embedding_lookup).

The reference computes, per drug-pair sample:
    multihot(drug) @ W0.T  ==  sum of W0 columns at the drug's (deduped)
    target proteins -- i.e. an embedding-table gather/sum, followed by a
    tiny MLP tower on each leg and a dot product between the two legs.

Structure (8 NeuronCores, data-parallel):
  Launch A: drugs sharded 500/core (padded to 512). Each core issues 4
      large dma_gathers (4096 idxs each, all on ONE SWDGE queue so they
      complete in FIFO order) of 512B bf16 rows of the transposed W0
      table (dups remapped to a zero row to preserve `.set` semantics)
      and tree-reduces each [128, 32, 256] tile to an E shard row block
      while later gathers still drain.
  Host:     concatenates the 8 E shards + 32 (cell + b0) rows into one
      lookup table E_ext [4128, 256] (pure data movement).
  Launch B: batch sharded 1024 samples/core, split in 2 groups of 512.
      Per group one transpose-mode gather pulls, in block layout,
      [E[d0] x512 | E[d1] x512 | cellb x512] feature-major; DVE adds
      e-leg + cellb (contiguous), ACT applies ReLU (b0 pre-folded into
      cellb), two matmul layers (W1, W2) and a ones-matmul pair-dot
      produce the [1024] outputs per core.
"""

import os

os.environ.setdefault("JAX_PLATFORMS", "")

import numpy as np
import ml_dtypes

import concourse.bacc as bacc
import concourse.mybir as mybir
from concourse.tile import TileContext
from concourse import library_config
from concourse.bass_utils import run_bass_kernel_spmd

# Problem constants (hardcoded per harness contract).
B = 8192            # samples
P = 19000           # proteins
C = 32              # cell lines
D = 4000            # drugs
T = 32              # targets per drug
F = 256             # first hidden dim
H1 = 128            # second hidden dim
H2 = 64             # output dim per tower

NCORES = 8
DRUGS_PER_CORE = D // NCORES          # 500
DRUGS_PAD = 512                       # per-core padded drug count
SAMPLES_PER_CORE = B // NCORES        # 1024
ZROW = P + C                          # zero row in the W0T table (19032)
TAB_ROWS = ZROW + 8                   # pad table rows to 19040
E_ROWS = NCORES * DRUGS_PAD           # 4096 rows of E
EXT_ROWS = E_ROWS + C                 # + 32 cell rows = 4128
NI_A = DRUGS_PAD * T                  # 16384 gather idxs per core, launch A
NI_B = 3 * SAMPLES_PER_CORE           # 3072 gather idxs per core, launch B
N_SUB = 4                             # launch A sub-batches of 128 drugs
NI_S = NI_A // N_SUB                  # 4096 idxs per launch-A gather
GRP = 2                               # launch B sample groups
SG = SAMPLES_PER_CORE // GRP          # 512 samples per group
NI_G = 3 * SG                         # 1536 idxs per launch-B gather
NQ = 4                                # SWDGE queues

_BF16 = mybir.dt.bfloat16
_F32 = mybir.dt.float32
_I16 = mybir.dt.int16

_cache = {}


def _wrap_idx(flat):
    """Flat gather order -> the [128, n/16] int16 SBUF layout dma_gather
    expects (idx i at partition i%16, slot i//16; replicated to all 8 Q7
    core slices)."""
    n = flat.shape[0]
    assert n % 16 == 0
    arr = flat.astype(np.int16).reshape(n // 16, 16).T.copy()
    return np.tile(arr, (8, 1))


NGA = 32                              # launch A gathers (512 idxs each)
NI_GA = NI_A // NGA                   # 512 idxs = 16 drugs per gather
DPG = NI_GA // T                      # 16 drugs per gather


def _build_kernel_a():
    nc = bacc.Bacc("TRN2", target_bir_lowering=True, num_swdge_queues=NQ)
    tab = nc.dram_tensor("tab", [TAB_ROWS, F], _BF16, kind="ExternalInput")
    idxs = nc.dram_tensor("idxs", [128, NI_A // 16], _I16, kind="ExternalInput")
    # feature-major E shard: e_out[p, c*512 + d] = E[drug d, feature c*128+p]
    e_out = nc.dram_tensor(
        "e_out", [128, 2 * DRUGS_PAD], _F32, kind="ExternalOutput")

    with TileContext(nc) as tc:
        nc.gpsimd.load_library(library_config.mlp)
        with (
            tc.tile_pool(name="idx", bufs=1) as ip,
            tc.tile_pool(name="g", bufs=1) as gp,
            tc.tile_pool(name="e", bufs=1) as ep,
        ):
            idx_t = ip.tile([128, NI_A // 16], _I16)
            nc.sync.dma_start(out=idx_t[:, :], in_=idxs[:, :])
            # transpose-mode gathers (columns pack ~24 descs/packet vs one
            # 512B packet per row in plain mode), spread over all 4 SWDGE
            # queues which drain in parallel (~53 GB/s each). Idx order is
            # drug-major so each gather holds 16 whole drugs and reduces
            # independently with a single DVE tensor_reduce.
            gs = []
            for i in range(NGA):
                g = gp.tile([128, 2, NI_GA], _BF16, tag=f"g{i}")
                nc.gpsimd.dma_gather(
                    g[:, :, :],
                    tab[:],
                    idx_t[:, i * (NI_GA // 16):(i + 1) * (NI_GA // 16)],
                    NI_GA, NI_GA, F,
                    transpose=True, single_packet=True, queue_num=i % NQ,
                )
                gs.append(g)
            e_feat = ep.tile([128, 2, DRUGS_PAD], _F32, tag="ef")
            ev = e_out.ap().rearrange("p (c d) -> p c d", c=2)
            for i in range(NGA):
                nc.vector.tensor_reduce(
                    out=e_feat[:, :, i * DPG:(i + 1) * DPG],
                    in_=gs[i][:, :, :].rearrange("p c (d t) -> p c d t", t=T),
                    axis=mybir.AxisListType.X,
                    op=mybir.AluOpType.add,
                )
                if i % 8 == 7:
                    b = i // 8
                    nc.sync.dma_start(
                        out=ev[:, :, b * 128:(b + 1) * 128],
                        in_=e_feat[:, :, b * 128:(b + 1) * 128],
                    )
    nc.compile()
    return nc


def _build_kernel_b():
    nc = bacc.Bacc("TRN2", target_bir_lowering=True, num_swdge_queues=NQ)
    etab = nc.dram_tensor("etab", [EXT_ROWS, F], _BF16, kind="ExternalInput")
    idxs = nc.dram_tensor("idxs", [128, NI_B // 16], _I16, kind="ExternalInput")
    w1t = nc.dram_tensor("w1t", [F, H1], _BF16, kind="ExternalInput")
    w2t = nc.dram_tensor("w2t", [H1, H2], _BF16, kind="ExternalInput")
    b1t = nc.dram_tensor("b1t", [128, 1], _F32, kind="ExternalInput")
    b2t = nc.dram_tensor("b2t", [64, 1], _F32, kind="ExternalInput")
    y = nc.dram_tensor("y", [1, SAMPLES_PER_CORE], _F32, kind="ExternalOutput")

    S = SAMPLES_PER_CORE                      # 1024
    L = 2 * S                                 # 2048 legs
    TN = 512                                  # matmul moving-tile width
    with TileContext(nc) as tc:
        nc.gpsimd.load_library(library_config.mlp)
        with (
            tc.tile_pool(name="const", bufs=1) as cp,
            tc.tile_pool(name="act", bufs=1) as ap,
            tc.tile_pool(name="ps", bufs=2, space="PSUM") as pp,
        ):
            idx_t = cp.tile([128, NI_B // 16], _I16)
            nc.sync.dma_start(out=idx_t[:, :], in_=idxs[:, :])
            # W1T is [256, H1]; SBUF partition dim is 128 -> [128, 2, H1]
            w1_t = cp.tile([128, 2, H1], _BF16, tag="w1")
            nc.sync.dma_start(
                out=w1_t[:, :, :],
                in_=w1t.ap().rearrange("(c p) h -> p c h", p=128),
            )
            w2_t = cp.tile([128, H2], _BF16, tag="w2")
            nc.sync.dma_start(out=w2_t[:, :], in_=w2t[:, :])
            b1_t = cp.tile([128, 1], _F32, tag="b1")
            nc.sync.dma_start(out=b1_t[:, :], in_=b1t[:, :])
            b2_t = cp.tile([64, 1], _F32, tag="b2")
            nc.sync.dma_start(out=b2_t[:, :], in_=b2t[:, :])
            ones = cp.tile([64, 1], _F32, tag="ones")
            nc.vector.memset(ones[:, :], 1.0)
            zero_b = cp.tile([128, 1], _F32, tag="zb")
            nc.vector.memset(zero_b[:, :], 0.0)

            # fused gather, 6 transpose-mode gathers of 512 idxs (large
            # single-packet transpose gathers hang the SDMA), per
            # 512-sample group h in role order [cellb, E[d0], E[d1]],
            # spread over the 4 parallel SWDGE queues.
            xg = [[None] * 3 for _ in range(GRP)]
            for h in range(GRP):
                for r in range(3):
                    xt = ap.tile([128, 2, SG], _BF16, tag=f"xt{h}{r}")
                    nc.gpsimd.dma_gather(
                        xt[:, :, :], etab[:],
                        idx_t[:, (3 * h + r) * (SG // 16):
                              (3 * h + r + 1) * (SG // 16)],
                        SG, SG, F,
                        transpose=True, single_packet=True,
                        queue_num=(3 * h + r) % NQ,
                    )
                    xg[h][r] = xt

            h0 = ap.tile([128, 2, L], _BF16, tag="h0")
            h1 = ap.tile([128, L], _BF16, tag="h1")
            h2 = ap.tile([64, L], _F32, tag="h2")
            prod = ap.tile([64, S], _F32, tag="prod")
            out_sb = ap.tile([1, S], _F32, tag="out")
            for h in range(GRP):
                hb = h * 2 * SG
                for leg in range(2):
                    nt = 2 * h + leg
                    # pre-activation: e-leg + cellb (b0 folded into cellb)
                    nc.vector.tensor_tensor(
                        out=h0[:, :, hb + leg * SG:hb + (leg + 1) * SG],
                        in0=xg[h][1 + leg][:, :, :],
                        in1=xg[h][0][:, :, :],
                        op=mybir.AluOpType.add,
                    )
                    for c in range(2):
                        nc.scalar.activation(
                            h0[:, c, hb + leg * SG:hb + (leg + 1) * SG],
                            h0[:, c, hb + leg * SG:hb + (leg + 1) * SG],
                            mybir.ActivationFunctionType.Relu,
                            bias=zero_b[:, 0:1], scale=1.0,
                        )
                    ps1 = pp.tile([128, TN], _F32, tag="ps1")
                    for c in range(2):
                        nc.tensor.matmul(
                            ps1[:, :], w1_t[:, c, :],
                            h0[:, c, nt * TN:(nt + 1) * TN],
                            start=(c == 0), stop=(c == 1),
                        )
                    nc.scalar.activation(
                        h1[:, nt * TN:(nt + 1) * TN], ps1[:, :],
                        mybir.ActivationFunctionType.Relu,
                        bias=b1_t[:, 0:1], scale=1.0,
                    )
                    ps2 = pp.tile([64, TN], _F32, tag="ps2")
                    nc.tensor.matmul(
                        ps2[:, :], w2_t[:, :], h1[:, nt * TN:(nt + 1) * TN],
                        start=True, stop=True,
                    )
                    # h2 = ps2 + b2 on DVE (keeps scalar engine all-Relu,
                    # avoiding activation-table reloads)
                    nc.vector.tensor_scalar(
                        out=h2[:, nt * TN:(nt + 1) * TN],
                        in0=ps2[:, :],
                        scalar1=b2_t[:, 0:1], scalar2=None,
                        op0=mybir.AluOpType.add,
                    )
                # pair product: leg0 block x leg1 block of this group
                nc.vector.tensor_tensor(
                    out=prod[:, h * SG:(h + 1) * SG],
                    in0=h2[:, hb:hb + SG],
                    in1=h2[:, hb + SG:hb + 2 * SG],
                    op=mybir.AluOpType.mult,
                )
                ps3 = pp.tile([1, SG], _F32, tag="ps3")
                nc.tensor.matmul(
                    ps3[:, :], ones[:, :], prod[:, h * SG:(h + 1) * SG],
                    start=True, stop=True,
                )
                nc.vector.tensor_copy(
                    out_sb[:, h * SG:(h + 1) * SG], ps3[:, :]
                )
            nc.sync.dma_start(out=y[:, :], in_=out_sb[:, :])
    nc.compile()
    return nc


def _get_kernels():
    if "a" not in _cache:
        _cache["a"] = _build_kernel_a()
    if "b" not in _cache:
        _cache["b"] = _build_kernel_b()
    return _cache["a"], _cache["b"]


def _prep(drug_pairs, cell_lines, drug_targets, W0, b0, W1, b1, W2, b2):
    """Host-side data layout: shard, transpose, cast, build gather indices."""
    dt = np.asarray(drug_targets, dtype=np.int64)                  # [D, T]
    # dedup per row (reference uses .set -> dup targets count once)
    dup = (dt[:, :, None] == dt[:, None, :]) & (
        np.arange(T)[None, :, None] > np.arange(T)[None, None, :]
    )
    idx = np.where(dup.any(-1), ZROW, dt).astype(np.int32)          # [D, T]

    # W0T table: [P+C rows, F] bf16 + zero row + pad
    w0t = np.zeros((TAB_ROWS, F), dtype=ml_dtypes.bfloat16)
    w0t[: P + C] = np.asarray(W0, np.float32).T.astype(ml_dtypes.bfloat16)

    # launch A per-core gather index arrays, drug-major (idx j -> drug
    # j//T target j%T): transpose-gather column j, 16 drugs per gather
    idx_a = []
    for c in range(NCORES):
        rows = np.full((DRUGS_PAD, T), ZROW, np.int32)
        rows[:DRUGS_PER_CORE] = idx[c * DRUGS_PER_CORE:(c + 1) * DRUGS_PER_CORE]
        idx_a.append(_wrap_idx(rows.reshape(-1)))

    # launch B per-core index arrays (built against E_ext layout), block
    # layout per 512-sample group: [e0 x512 | e1 x512 | cellb x512]
    dp = np.asarray(drug_pairs, dtype=np.int64)                     # [B, 2]
    cl = np.asarray(cell_lines, dtype=np.int64)                     # [B]
    e_row = (dp // DRUGS_PER_CORE) * DRUGS_PAD + (dp % DRUGS_PER_CORE)
    cell_row = E_ROWS + cl
    idx_b = []
    for c in range(NCORES):
        sl = slice(c * SAMPLES_PER_CORE, (c + 1) * SAMPLES_PER_CORE)
        e0, e1, cr = e_row[sl, 0], e_row[sl, 1], cell_row[sl]
        parts = []
        for h in range(GRP):
            g = slice(h * SG, (h + 1) * SG)
            parts += [cr[g], e0[g], e1[g]]
        idx_b.append(_wrap_idx(np.concatenate(parts)))

    w1t = np.ascontiguousarray(
        np.asarray(W1, np.float32).T.astype(ml_dtypes.bfloat16))    # [F, H1]
    w2t = np.ascontiguousarray(
        np.asarray(W2, np.float32).T.astype(ml_dtypes.bfloat16))    # [H1, H2]
    b1t = np.asarray(b1, np.float32).reshape(128, 1).copy()
    b2t = np.asarray(b2, np.float32).reshape(64, 1).copy()
    # cellb rows: W0 cell columns + b0, so launch B needs no separate bias
    celltab = (
        np.asarray(W0, np.float32)[:, P:P + C].T
        + np.asarray(b0, np.float32)[None, :]
    ).astype(ml_dtypes.bfloat16)                                    # [C, F]
    return w0t, idx_a, idx_b, w1t, w2t, b1t, b2t, celltab


def _run(inputs, trace=False):
    nca, ncb = _get_kernels()
    w0t, idx_a, idx_b, w1t, w2t, b1t, b2t, celltab = _prep(**inputs)

    in_a = [{"tab": w0t, "idxs": idx_a[c]} for c in range(NCORES)]
    res_a = run_bass_kernel_spmd(
        nca, in_a, core_ids=list(range(NCORES)), trace=trace)

    # e_out[p, c, d] (feature-major) -> row-major [512, 256] per core
    shards = []
    for c in range(NCORES):
        eo = np.asarray(res_a.results[c]["e_out"], np.float32)
        eo = eo.reshape(128, 2, DRUGS_PAD).transpose(2, 1, 0).reshape(
            DRUGS_PAD, F)
        shards.append(eo.astype(ml_dtypes.bfloat16))
    e_ext = np.concatenate(shards + [celltab], axis=0)
    assert e_ext.shape == (EXT_ROWS, F)

    in_b = [
        {"etab": e_ext, "idxs": idx_b[c], "w1t": w1t, "w2t": w2t,
         "b1t": b1t, "b2t": b2t}
        for c in range(NCORES)
    ]
    res_b = run_bass_kernel_spmd(
        ncb, in_b, core_ids=list(range(NCORES)), trace=trace)
    _cache["res_a"], _cache["res_b"] = res_a, res_b

    out = np.concatenate(
        [res_b.results[c]["y"].reshape(-1) for c in range(NCORES)]
    ).astype(np.float32)
    times = (res_a.exec_time_ns, res_b.exec_time_ns)
    return out, times


def kernel(**inputs) -> np.ndarray:
    out, _ = _run(inputs, trace=False)
    return out
